# revision 2
# baseline (speedup 1.0000x reference)
"""Alpha-filter (keras_spiking AlphaCell) Trainium2 Bass kernel.

Math: per (batch b, feature k) the reference runs the 2-state recurrence
    x_t = A_k x_{t-1} + B_k u_t,   y_t = x_t[1]
with A_k = e*[[1-a, -a/tau],[dt, 1+a]], a = dt/tau, e = exp(-a).
A_k has a defective double eigenvalue e (A = e(I+N), N nilpotent), so the
recurrence splits into two chained first-order scans plus pointwise ops:

    s_t   = e * s_{t-1} + u_t            s_0 = L / (1-e)      (L = initial_level)
    eta_t = e * eta_{t-1} + (s_{t-1} - s_t)                   eta_0 = 0
    y_t   = (e*a) * eta_t + (1-e) * s_t

Each first-order scan maps to one DVE tensor_tensor_scan over a
[128 features, T] tile.  The final combine is folded into the PE
transpose-back: y[t,k] = sum_p eta[p,t]*diag(e*a)[p,k] + s[p,t]*diag(1-e)[p,k]
as two accumulating matmuls per 128x128 block.

Sharding: data-parallel over batch, 8 batches per core x 8 cores.
"""

import sys

for _p in ("/opt/trn_rl_repo",):
    if _p not in sys.path:
        sys.path.insert(0, _p)

from contextlib import ExitStack

import numpy as np

import concourse.bacc as bacc
import concourse.bass as bass
import concourse.tile as tile
from concourse import mybir
from concourse.bass_utils import run_bass_kernel_spmd

DT = 0.001
B, T, K = 64, 1024, 512
N_CORES = 8
B_LOC = B // N_CORES  # 8 batches per core
P = 128
KC = K // P   # 4 feature chunks of 128
TCH = T // P  # 8 time chunks of 128

F32 = mybir.dt.float32
MULT = mybir.AluOpType.mult
ADD = mybir.AluOpType.add


def build_nc():
    nc = bacc.Bacc(None, target_bir_lowering=False)

    x = nc.dram_tensor("x", [B_LOC, T, K], F32, kind="ExternalInput")
    e_c = nc.dram_tensor("e_c", [KC, P], F32, kind="ExternalInput")
    s0_c = nc.dram_tensor("s0_c", [KC, P], F32, kind="ExternalInput")
    # dmat[c, 0] = diag(e*a) chunk, dmat[c, 1] = diag(1-e) chunk
    dmat = nc.dram_tensor("dmat", [KC, 2, P, P], F32, kind="ExternalInput")
    ident = nc.dram_tensor("ident", [P, P], F32, kind="ExternalInput")
    y = nc.dram_tensor("y", [B_LOC, T, K], F32, kind="ExternalOutput")

    with tile.TileContext(nc) as tc, ExitStack() as ctx:
        singles = ctx.enter_context(tc.tile_pool(name="singles", bufs=1))
        inpool = ctx.enter_context(tc.tile_pool(name="inpool", bufs=2))
        outpool = ctx.enter_context(tc.tile_pool(name="outpool", bufs=2))
        spool = ctx.enter_context(tc.tile_pool(name="spool", bufs=2 * KC))
        epool = ctx.enter_context(tc.tile_pool(name="epool", bufs=2 * KC))
        vpool = ctx.enter_context(tc.tile_pool(name="vpool", bufs=3))
        psum_u = ctx.enter_context(tc.tile_pool(name="psum_u", bufs=2, space="PSUM"))
        psum_y = ctx.enter_context(tc.tile_pool(name="psum_y", bufs=2, space="PSUM"))

        # ---- one-time constant loads -----------------------------------
        e_col = singles.tile([P, KC], F32)
        nc.sync.dma_start(out=e_col[:], in_=e_c.rearrange("c p -> p c"))
        s0_col = singles.tile([P, KC], F32)
        nc.sync.dma_start(out=s0_col[:], in_=s0_c.rearrange("c p -> p c"))
        ident_t = singles.tile([P, P], F32)
        nc.sync.dma_start(out=ident_t[:], in_=ident[:])
        dea_t = []
        dem1_t = []
        for c in range(KC):
            da = singles.tile([P, P], F32, tag=f"dea{c}")
            nc.sync.dma_start(out=da[:], in_=dmat[c, 0])
            dea_t.append(da)
            dm = singles.tile([P, P], F32, tag=f"dem1{c}")
            nc.sync.dma_start(out=dm[:], in_=dmat[c, 1])
            dem1_t.append(dm)

        # e broadcast along free dim: scan multiplier tiles
        e_tiles = []
        for c in range(KC):
            et = singles.tile([P, T], F32, tag=f"etile{c}")
            nc.gpsimd.memset(et[:], 0.0)
            nc.gpsimd.tensor_scalar_add(et[:], et[:], e_col[:, c : c + 1])
            e_tiles.append(et)

        # ---- main loop over local batches ------------------------------
        for b in range(B_LOC):
            # staged input: in_stage[p, tch, k] = x[b, tch*128+p, k]
            in_stage = inpool.tile([P, TCH, K], F32)
            nc.sync.dma_start(
                out=in_stage[:], in_=x[b].rearrange("(a p) k -> p a k", p=P)
            )

            s_tiles = []
            eta_tiles = []
            for c in range(KC):
                # transpose u into [128 features, T] (PSUM), time along free
                uT = psum_u.tile([P, T], F32)
                for t in range(TCH):
                    nc.tensor.transpose(
                        uT[:, t * P : (t + 1) * P],
                        in_stage[:, t, c * P : (c + 1) * P],
                        ident_t[:],
                    )

                s_full = spool.tile([P, T + 1], F32)
                nc.scalar.copy(s_full[:, 0:1], s0_col[:, c : c + 1])
                nc.vector.tensor_tensor_scan(
                    out=s_full[:, 1 : T + 1],
                    data0=e_tiles[c][:],
                    data1=uT[:],
                    initial=s0_col[:, c : c + 1],
                    op0=MULT,
                    op1=ADD,
                )

                v = vpool.tile([P, T], F32)
                nc.gpsimd.tensor_sub(v[:], s_full[:, 0:T], s_full[:, 1 : T + 1])

                eta = epool.tile([P, T], F32)
                nc.vector.tensor_tensor_scan(
                    out=eta[:],
                    data0=e_tiles[c][:],
                    data1=v[:],
                    initial=0.0,
                    op0=MULT,
                    op1=ADD,
                )
                s_tiles.append(s_full)
                eta_tiles.append(eta)

            # transpose back with the per-feature combine folded into PE:
            # y[t, k] = (e a)_k eta[k, t] + (1-e)_k s[k, t]
            out_stage = outpool.tile([P, TCH, K], F32)
            for t in range(TCH):
                yp = psum_y.tile([P, K], F32)
                for c in range(KC):
                    nc.tensor.matmul(
                        yp[:, c * P : (c + 1) * P],
                        eta_tiles[c][:, t * P : (t + 1) * P],
                        dea_t[c][:],
                        start=True,
                        stop=False,
                    )
                    nc.tensor.matmul(
                        yp[:, c * P : (c + 1) * P],
                        s_tiles[c][:, 1 + t * P : 1 + (t + 1) * P],
                        dem1_t[c][:],
                        start=False,
                        stop=True,
                    )
                nc.scalar.copy(out_stage[:, t, :], yp[:])

            nc.sync.dma_start(
                out=y[b].rearrange("(a p) k -> p a k", p=P), in_=out_stage[:]
            )

    nc.compile()
    return nc


_CACHE = {}
PROFILE = False
LAST_RESULT = None


def _host_constants(initial_level, tau):
    tau_c = np.maximum(tau.astype(np.float64), 1e-8)
    a = DT / tau_c
    e = np.exp(-a)
    em1 = 1.0 - e
    ea = e * a
    s0 = initial_level.astype(np.float64) / em1
    e_c = e.astype(np.float32).reshape(KC, P)
    s0_c = s0.astype(np.float32).reshape(KC, P)
    dmat = np.zeros((KC, 2, P, P), dtype=np.float32)
    for c in range(KC):
        np.fill_diagonal(dmat[c, 0], ea[c * P : (c + 1) * P].astype(np.float32))
        np.fill_diagonal(dmat[c, 1], em1[c * P : (c + 1) * P].astype(np.float32))
    ident = np.eye(P, dtype=np.float32)
    return e_c, s0_c, dmat, ident


def kernel(inputs, initial_level, tau):
    global LAST_RESULT
    inputs = np.ascontiguousarray(np.asarray(inputs, dtype=np.float32))
    initial_level = np.asarray(initial_level, dtype=np.float32)
    tau = np.asarray(tau, dtype=np.float32)
    assert inputs.shape == (B, T, K), inputs.shape

    e_c, s0_c, dmat, ident = _host_constants(initial_level, tau)

    if "nc" not in _CACHE:
        _CACHE["nc"] = build_nc()
    nc = _CACHE["nc"]

    in_maps = [
        {
            "x": inputs[i * B_LOC : (i + 1) * B_LOC],
            "e_c": e_c,
            "s0_c": s0_c,
            "dmat": dmat,
            "ident": ident,
        }
        for i in range(N_CORES)
    ]
    res = run_bass_kernel_spmd(nc, in_maps, list(range(N_CORES)), trace=PROFILE)
    LAST_RESULT = res
    return np.concatenate([r["y"] for r in res.results], axis=0)


# revision 3
# speedup vs baseline: 1.2278x; 1.2278x over previous
"""Alpha-filter (keras_spiking AlphaCell) Trainium2 Bass kernel.

Math: per (batch b, feature k) the reference runs the 2-state recurrence
    x_t = A_k x_{t-1} + B_k u_t,   y_t = x_t[1]
with A_k = e*[[1-a, -a/tau],[dt, 1+a]], a = dt/tau, e = exp(-a).
A_k has a defective double eigenvalue e (A = e(I+N), N nilpotent), so the
recurrence reduces to two CHAINED first-order scans (scan2 consumes scan1's
output directly — no intermediate tensor):

    s_t   = e * s_{t-1} + u_t          s_0   = L/(1-e)      (L = initial_level)
    eta_t = e * eta_{t-1} + s_{t-1}    eta_0 = L/(1-e)^2
    y_t   = [e*a*(1-e)] * eta_t + [(1-e) - e*a] * s_t

Each scan is one DVE tensor_tensor_scan over a [128 features, T] tile
(time on the free dim).  Input tiles arrive in [time, features] layout
(contiguous DMA) and are transposed on the PE into PSUM; the scan reads
PSUM directly.  The final combine is folded into the PE transpose-back:
two accumulating matmuls against per-feature diagonal matrices produce
y in [time, features] layout in PSUM; ScalarE copies to SBUF for the
contiguous store.

Sharding: data-parallel over batch, 8 batches per core x 8 cores.
"""

import sys

for _p in ("/opt/trn_rl_repo",):
    if _p not in sys.path:
        sys.path.insert(0, _p)

from contextlib import ExitStack

import numpy as np

import concourse.bacc as bacc
import concourse.bass as bass
import concourse.tile as tile
from concourse import mybir
from concourse.bass_utils import run_bass_kernel_spmd

DT = 0.001
B, T, K = 64, 1024, 512
N_CORES = 8
B_LOC = B // N_CORES  # 8 batches per core
P = 128
KC = K // P   # 4 feature chunks of 128
TCH = T // P  # 8 time chunks of 128

F32 = mybir.dt.float32
MULT = mybir.AluOpType.mult
ADD = mybir.AluOpType.add


def build_nc():
    nc = bacc.Bacc(None, target_bir_lowering=False)

    x = nc.dram_tensor("x", [B_LOC, T, K], F32, kind="ExternalInput")
    # cols[c] = [e, s0, eta0] per feature chunk
    cols = nc.dram_tensor("cols", [KC, 3, P], F32, kind="ExternalInput")
    # dmat[c, 0] = diag(e*a*(1-e)) chunk, dmat[c, 1] = diag((1-e) - e*a)
    dmat = nc.dram_tensor("dmat", [KC, 2, P, P], F32, kind="ExternalInput")
    ident = nc.dram_tensor("ident", [P, P], F32, kind="ExternalInput")
    y = nc.dram_tensor("y", [B_LOC, T, K], F32, kind="ExternalOutput")

    with tile.TileContext(nc) as tc, ExitStack() as ctx:
        singles = ctx.enter_context(tc.tile_pool(name="singles", bufs=1))
        inpool = ctx.enter_context(tc.tile_pool(name="inpool", bufs=2))
        outpool = ctx.enter_context(tc.tile_pool(name="outpool", bufs=2))
        spool = ctx.enter_context(tc.tile_pool(name="spool", bufs=2 * KC))
        epool = ctx.enter_context(tc.tile_pool(name="epool", bufs=2 * KC))
        psum_u = ctx.enter_context(tc.tile_pool(name="psum_u", bufs=2, space="PSUM"))
        psum_y = ctx.enter_context(tc.tile_pool(name="psum_y", bufs=4, space="PSUM"))

        # ---- one-time constant loads -----------------------------------
        e_col = singles.tile([P, KC], F32)
        nc.sync.dma_start(out=e_col[:], in_=cols.rearrange("c s p -> p c s")[:, :, 0])
        s0_col = singles.tile([P, KC], F32)
        nc.sync.dma_start(out=s0_col[:], in_=cols.rearrange("c s p -> p c s")[:, :, 1])
        eta0_col = singles.tile([P, KC], F32)
        nc.sync.dma_start(
            out=eta0_col[:], in_=cols.rearrange("c s p -> p c s")[:, :, 2]
        )
        ident_t = singles.tile([P, P], F32)
        nc.sync.dma_start(out=ident_t[:], in_=ident[:])
        dea_t = []
        dem1_t = []
        for c in range(KC):
            da = singles.tile([P, P], F32, tag=f"dea{c}")
            nc.sync.dma_start(out=da[:], in_=dmat[c, 0])
            dea_t.append(da)
            dm = singles.tile([P, P], F32, tag=f"dem1{c}")
            nc.sync.dma_start(out=dm[:], in_=dmat[c, 1])
            dem1_t.append(dm)

        # e broadcast along free dim: scan multiplier tiles (one-time, DVE)
        e_tiles = []
        for c in range(KC):
            et = singles.tile([P, T], F32, tag=f"etile{c}")
            nc.vector.memset(et[:], 0.0)
            nc.vector.tensor_scalar_add(et[:], et[:], e_col[:, c : c + 1])
            e_tiles.append(et)

        # ---- main loop over local batches ------------------------------
        for b in range(B_LOC):
            # staged input: in_stage[p, tch, k] = x[b, tch*128+p, k]
            in_stage = inpool.tile([P, TCH, K], F32)
            nc.sync.dma_start(
                out=in_stage[:], in_=x[b].rearrange("(a p) k -> p a k", p=P)
            )

            s_tiles = []
            eta_tiles = []
            for c in range(KC):
                # transpose u into [128 features, T] (PSUM), time along free
                uT = psum_u.tile([P, T], F32)
                for t in range(TCH):
                    nc.tensor.transpose(
                        uT[:, t * P : (t + 1) * P],
                        in_stage[:, t, c * P : (c + 1) * P],
                        ident_t[:],
                    )

                s_full = spool.tile([P, T + 1], F32)
                nc.scalar.copy(s_full[:, 0:1], s0_col[:, c : c + 1])
                nc.vector.tensor_tensor_scan(
                    out=s_full[:, 1 : T + 1],
                    data0=e_tiles[c][:],
                    data1=uT[:],
                    initial=s0_col[:, c : c + 1],
                    op0=MULT,
                    op1=ADD,
                )

                eta = epool.tile([P, T], F32)
                nc.vector.tensor_tensor_scan(
                    out=eta[:],
                    data0=e_tiles[c][:],
                    data1=s_full[:, 0:T],
                    initial=eta0_col[:, c : c + 1],
                    op0=MULT,
                    op1=ADD,
                )
                s_tiles.append(s_full)
                eta_tiles.append(eta)

            # transpose back with the per-feature combine folded into PE:
            # y[t, k] = [e a (1-e)]_k eta[k, t] + [(1-e) - e a]_k s[k, t]
            out_stage = outpool.tile([P, TCH, K], F32)
            for t in range(TCH):
                yp = psum_y.tile([P, K], F32)
                for c in range(KC):
                    nc.tensor.matmul(
                        yp[:, c * P : (c + 1) * P],
                        eta_tiles[c][:, t * P : (t + 1) * P],
                        dea_t[c][:],
                        start=True,
                        stop=False,
                    )
                    nc.tensor.matmul(
                        yp[:, c * P : (c + 1) * P],
                        s_tiles[c][:, 1 + t * P : 1 + (t + 1) * P],
                        dem1_t[c][:],
                        start=False,
                        stop=True,
                    )
                nc.scalar.copy(out_stage[:, t, :], yp[:])

            nc.sync.dma_start(
                out=y[b].rearrange("(a p) k -> p a k", p=P), in_=out_stage[:]
            )

    nc.compile()
    return nc


_CACHE = {}
PROFILE = False
LAST_RESULT = None


def _host_constants(initial_level, tau):
    tau_c = np.maximum(tau.astype(np.float64), 1e-8)
    a = DT / tau_c
    e = np.exp(-a)
    em1 = 1.0 - e
    ea = e * a
    s0 = initial_level.astype(np.float64) / em1
    eta0 = initial_level.astype(np.float64) / (em1 * em1)
    c_eta = ea * em1
    c_s = em1 - ea
    cols = np.stack(
        [
            e.astype(np.float32).reshape(KC, P),
            s0.astype(np.float32).reshape(KC, P),
            eta0.astype(np.float32).reshape(KC, P),
        ],
        axis=1,
    )  # [KC, 3, P]
    dmat = np.zeros((KC, 2, P, P), dtype=np.float32)
    for c in range(KC):
        np.fill_diagonal(dmat[c, 0], c_eta[c * P : (c + 1) * P].astype(np.float32))
        np.fill_diagonal(dmat[c, 1], c_s[c * P : (c + 1) * P].astype(np.float32))
    ident = np.eye(P, dtype=np.float32)
    return cols, dmat, ident


def kernel(inputs, initial_level, tau):
    global LAST_RESULT
    inputs = np.ascontiguousarray(np.asarray(inputs, dtype=np.float32))
    initial_level = np.asarray(initial_level, dtype=np.float32)
    tau = np.asarray(tau, dtype=np.float32)
    assert inputs.shape == (B, T, K), inputs.shape

    cols, dmat, ident = _host_constants(initial_level, tau)

    if "nc" not in _CACHE:
        _CACHE["nc"] = build_nc()
    nc = _CACHE["nc"]

    in_maps = [
        {
            "x": inputs[i * B_LOC : (i + 1) * B_LOC],
            "cols": cols,
            "dmat": dmat,
            "ident": ident,
        }
        for i in range(N_CORES)
    ]
    res = run_bass_kernel_spmd(nc, in_maps, list(range(N_CORES)), trace=PROFILE)
    LAST_RESULT = res
    return np.concatenate([r["y"] for r in res.results], axis=0)


# revision 4
# speedup vs baseline: 1.5783x; 1.2855x over previous
"""Alpha-filter (keras_spiking AlphaCell) Trainium2 Bass kernel.

Math: per (batch b, feature k) the reference runs the 2-state recurrence
    x_t = A_k x_{t-1} + B_k u_t,   y_t = x_t[1]
with A_k = e*[[1-a, -a/tau],[dt, 1+a]], a = dt/tau, e = exp(-a).
A_k has a defective double eigenvalue e (A = e(I+N), N nilpotent), so the
recurrence reduces to two CHAINED first-order scans (scan2 consumes scan1's
output directly — no intermediate tensor):

    s_t   = e * s_{t-1} + u_t          s_0   = L/(1-e)      (L = initial_level)
    eta_t = e * eta_{t-1} + s_{t-1}    eta_0 = L/(1-e)^2
    y_t   = [e*a*(1-e)] * eta_t + [(1-e) - e*a] * s_t

Each scan is one DVE tensor_tensor_scan over a [128 features, T] tile
(time on the free dim).  Input tiles arrive in [time, features] layout
(contiguous DMA) and are transposed on the PE into PSUM; the scan reads
PSUM directly.  The final combine is folded into the PE transpose-back:
two accumulating matmuls against per-feature diagonal matrices produce
y in [time, features] layout in PSUM; ScalarE copies to SBUF for the
contiguous store.

Sharding: data-parallel over batch, 8 batches per core x 8 cores.
"""

import sys

for _p in ("/opt/trn_rl_repo",):
    if _p not in sys.path:
        sys.path.insert(0, _p)

from contextlib import ExitStack

import numpy as np

import concourse.bacc as bacc
import concourse.bass as bass
import concourse.tile as tile
from concourse import mybir
from concourse.bass_utils import run_bass_kernel_spmd

DT = 0.001
B, T, K = 64, 1024, 512
N_CORES = 8
B_LOC = B // N_CORES  # 8 batches per core
P = 128
KC = K // P   # 4 feature chunks of 128
TCH = T // P  # 8 time chunks of 128

F32 = mybir.dt.float32
MULT = mybir.AluOpType.mult
ADD = mybir.AluOpType.add


def build_nc():
    nc = bacc.Bacc(None, target_bir_lowering=False)

    x = nc.dram_tensor("x", [B_LOC, T, K], F32, kind="ExternalInput")
    # cols[c] = [e, s0, eta0, c_eta, c_s] per feature chunk
    cols = nc.dram_tensor("cols", [KC, 5, P], F32, kind="ExternalInput")
    ident = nc.dram_tensor("ident", [P, P], F32, kind="ExternalInput")
    y = nc.dram_tensor("y", [B_LOC, T, K], F32, kind="ExternalOutput")

    with tile.TileContext(nc) as tc, ExitStack() as ctx:
        singles = ctx.enter_context(tc.tile_pool(name="singles", bufs=1))
        inpool = ctx.enter_context(tc.tile_pool(name="inpool", bufs=2))
        outpool = ctx.enter_context(tc.tile_pool(name="outpool", bufs=2))
        spool = ctx.enter_context(tc.tile_pool(name="spool", bufs=2 * KC))
        epool = ctx.enter_context(tc.tile_pool(name="epool", bufs=2 * KC))
        psum_u = ctx.enter_context(tc.tile_pool(name="psum_u", bufs=3, space="PSUM"))
        psum_y = ctx.enter_context(tc.tile_pool(name="psum_y", bufs=2, space="PSUM"))

        # ---- one-time constant loads -----------------------------------
        e_col = singles.tile([P, KC], F32)
        nc.sync.dma_start(out=e_col[:], in_=cols.rearrange("c s p -> p c s")[:, :, 0])
        s0_col = singles.tile([P, KC], F32)
        nc.sync.dma_start(out=s0_col[:], in_=cols.rearrange("c s p -> p c s")[:, :, 1])
        eta0_col = singles.tile([P, KC], F32)
        nc.sync.dma_start(
            out=eta0_col[:], in_=cols.rearrange("c s p -> p c s")[:, :, 2]
        )
        ceta_col = singles.tile([P, KC], F32)
        nc.sync.dma_start(
            out=ceta_col[:], in_=cols.rearrange("c s p -> p c s")[:, :, 3]
        )
        cs_col = singles.tile([P, KC], F32)
        nc.sync.dma_start(out=cs_col[:], in_=cols.rearrange("c s p -> p c s")[:, :, 4])
        ident_t = singles.tile([P, P], F32)
        nc.sync.dma_start(out=ident_t[:], in_=ident[:])

        # e broadcast along free dim: scan multiplier tiles (one-time, DVE)
        e_tiles = []
        for c in range(KC):
            et = singles.tile([P, T], F32, tag=f"etile{c}")
            nc.vector.memset(et[:], 0.0)
            nc.vector.tensor_scalar_add(et[:], et[:], e_col[:, c : c + 1])
            e_tiles.append(et)

        # ---- main loop over local batches ------------------------------
        for b in range(B_LOC):
            # staged input: in_stage[p, tch, k] = x[b, tch*128+p, k]
            in_stage = inpool.tile([P, TCH, K], F32)
            nc.sync.dma_start(
                out=in_stage[:], in_=x[b].rearrange("(a p) k -> p a k", p=P)
            )

            s_tiles = []
            eta_tiles = []
            for c in range(KC):
                # transpose u into [128 features, T] (PSUM), time along free
                uT = psum_u.tile([P, T], F32)
                for t in range(TCH):
                    nc.tensor.transpose(
                        uT[:, t * P : (t + 1) * P],
                        in_stage[:, t, c * P : (c + 1) * P],
                        ident_t[:],
                    )

                s_full = spool.tile([P, T + 1], F32)
                nc.scalar.copy(s_full[:, 0:1], s0_col[:, c : c + 1])
                nc.vector.tensor_tensor_scan(
                    out=s_full[:, 1 : T + 1],
                    data0=e_tiles[c][:],
                    data1=uT[:],
                    initial=s0_col[:, c : c + 1],
                    op0=MULT,
                    op1=ADD,
                )

                eta = epool.tile([P, T], F32)
                nc.vector.tensor_tensor_scan(
                    out=eta[:],
                    data0=e_tiles[c][:],
                    data1=s_full[:, 0:T],
                    initial=eta0_col[:, c : c + 1],
                    op0=MULT,
                    op1=ADD,
                )
                # in-place pre-scales on ScalarE: eta <- c_eta*eta,
                # s[:,1:] <- c_s*s[:,1:]  (scan2 already consumed s[:,0:T])
                nc.scalar.mul(eta[:], eta[:], ceta_col[:, c : c + 1])
                nc.scalar.mul(
                    s_full[:, 1 : T + 1],
                    s_full[:, 1 : T + 1],
                    cs_col[:, c : c + 1],
                )
                s_tiles.append(s_full)
                eta_tiles.append(eta)

            # transpose back; the combine is the PSUM accumulation of the
            # two pre-scaled transposes: y[t,k] = eta_hat[k,t] + s_hat[k,t]
            out_stage = outpool.tile([P, TCH, K], F32)
            for t in range(TCH):
                yp = psum_y.tile([P, K], F32)
                for c in range(KC):
                    nc.tensor.matmul(
                        yp[:, c * P : (c + 1) * P],
                        eta_tiles[c][:, t * P : (t + 1) * P],
                        ident_t[:],
                        is_transpose=True,
                        start=True,
                        stop=False,
                    )
                    nc.tensor.matmul(
                        yp[:, c * P : (c + 1) * P],
                        s_tiles[c][:, 1 + t * P : 1 + (t + 1) * P],
                        ident_t[:],
                        is_transpose=True,
                        start=False,
                        stop=True,
                    )
                nc.scalar.copy(out_stage[:, t, :], yp[:])

            nc.sync.dma_start(
                out=y[b].rearrange("(a p) k -> p a k", p=P), in_=out_stage[:]
            )

    nc.compile()
    return nc


_CACHE = {}
PROFILE = False
LAST_RESULT = None


def _host_constants(initial_level, tau):
    tau_c = np.maximum(tau.astype(np.float64), 1e-8)
    a = DT / tau_c
    e = np.exp(-a)
    em1 = 1.0 - e
    ea = e * a
    s0 = initial_level.astype(np.float64) / em1
    eta0 = initial_level.astype(np.float64) / (em1 * em1)
    c_eta = ea * em1
    c_s = em1 - ea
    cols = np.stack(
        [
            e.astype(np.float32).reshape(KC, P),
            s0.astype(np.float32).reshape(KC, P),
            eta0.astype(np.float32).reshape(KC, P),
            c_eta.astype(np.float32).reshape(KC, P),
            c_s.astype(np.float32).reshape(KC, P),
        ],
        axis=1,
    )  # [KC, 5, P]
    ident = np.eye(P, dtype=np.float32)
    return cols, ident


def kernel(inputs, initial_level, tau):
    global LAST_RESULT
    inputs = np.ascontiguousarray(np.asarray(inputs, dtype=np.float32))
    initial_level = np.asarray(initial_level, dtype=np.float32)
    tau = np.asarray(tau, dtype=np.float32)
    assert inputs.shape == (B, T, K), inputs.shape

    cols, ident = _host_constants(initial_level, tau)

    if "nc" not in _CACHE:
        _CACHE["nc"] = build_nc()
    nc = _CACHE["nc"]

    in_maps = [
        {
            "x": inputs[i * B_LOC : (i + 1) * B_LOC],
            "cols": cols,
            "ident": ident,
        }
        for i in range(N_CORES)
    ]
    res = run_bass_kernel_spmd(nc, in_maps, list(range(N_CORES)), trace=PROFILE)
    LAST_RESULT = res
    return np.concatenate([r["y"] for r in res.results], axis=0)
